# revision 8
# baseline (speedup 1.0000x reference)
"""Trainium2 Bass kernel for nn_LIFSNN (v2.1): T-sharded speculative scan.

Sharding: the sequential T=512 LIF scan is sharded across the 8 cores
in TIME (64 steps each) with a 24-step speculative warm-up from zero
state — the LIF soft-reset dynamics contract at rate beta<=0.73, so
the state converges exactly (numpy: 0 spike flips at W=32; W=24 device-validated). Each core
processes the FULL batch B=128 per step (96 steps/layer at FD=512
instead of 512 steps at FD=64).

Core 0 has no predecessor: its warm-up fold coefficients are
alpha-masked to zero, so its state stays exactly 0 through warm-up and
the main window starts from the true initial condition.

Precision (device-measured vs 2e-2 gate): matmuls f16 (full PE rate),
y and scan-1 state f32, scan-2 state f16 (its spike flips feed only
the smooth readout), spikes as exact +/-1 f16 with W/2 weights.

Per layer: matmul (PE) -> psum drain + stat accum (ACT) -> single
AllReduce of main-window stats (split ARs serialize on the collective
engine and lose) -> BN fold on ACT (into the scan's
input tiles) -> 2-chain 6-op DVE recurrence, spike Sign batched on
ACT (double-buffered U slots) into an f16 ring -> next layer's
matmuls interleaved 2 blocks at a time on PE. Readout: native
tensor_tensor_scan + PE-packed softmax. Host sums per-core partials.
"""
import sys, os, time

sys.path.insert(0, "/opt/trn_rl_repo")

import numpy as np
import ml_dtypes

import concourse.bass as bass
import concourse.mybir as mybir
import concourse.tile as tile
import concourse.bacc as bacc
from concourse.bass_utils import run_bass_kernel_spmd

AOT = mybir.AluOpType
AFT = mybir.ActivationFunctionType
F32 = mybir.dt.float32
F16 = mybir.dt.float16
BF16 = mybir.dt.bfloat16

NC = 8
B, T, J, H, O = 128, 512, 700, 512, 20
SH = T // NC
WU = 24
TL = SH + WU          # 96 local steps
NCOL = TL * B         # 12288
NBLK = NCOL // 512    # 24
NBW = WU * B // 512   # 8 warm-up blocks (scan order 0..7)
NBM = SH * B // 512   # 16 main blocks (8..23)
TBLK = 4
NCHUNK = H // 128
KJ = [(k * 128, min(128, J - k * 128)) for k in range((J + 127) // 128)]
NTOT = float(B * T)
EPS = 1e-5
SRING = 4096
NSPL = 12             # AR split: a covers main blocks [0,12), b [12,16)

CFG = "v21"
if CFG == "v21":       # scan1 f32, scan2 f16
    ST = [F32, F16]
elif CFG == "v21s":    # both f32
    ST = [F32, F32]
elif CFG == "v21f":    # both f16
    ST = [F16, F16]
else:
    raise ValueError(CFG)
YDT = [F32, F32]
NPARL = [8 if st == F32 else 16 for st in ST]
SWL = [n // 2 for n in NPARL]  # sign window (steps), double-buffered

_CACHE = {}


def _build_program():
    nc = bacc.Bacc("TRN2", target_bir_lowering=False, debug=False,
                   num_devices=NC)
    ap = lambda name, shape, dt, kind: nc.dram_tensor(name, shape, dt, kind=kind).ap()

    x_d = ap("x", [J, NCOL], F16, "ExternalInput")
    w1t_d = ap("w1t", [J, H], F16, "ExternalInput")
    w2t_d = ap("w2t", [H, H], F16, "ExternalInput")
    wrt_d = ap("wrt", [H, O], F16, "ExternalInput")
    nbf1_d = ap("nbf1", [128, NCHUNK * B], mybir.dt.float32 if ST[0] == F32 else F16, "ExternalInput")
    nbf2_d = ap("nbf2", [128, NCHUNK * B], mybir.dt.float32 if ST[1] == F32 else F16, "ExternalInput")
    alpha_d = ap("alpha", [128, 1], F32, "ExternalInput")
    G1_d = ap("G1", [128, NCHUNK], F32, "ExternalInput")
    Bb1_d = ap("Bb1", [128, NCHUNK], F32, "ExternalInput")
    G2_d = ap("G2", [128, NCHUNK], F32, "ExternalInput")
    Bb2_d = ap("Bb2", [128, NCHUNK], F32, "ExternalInput")
    brb_d = ap("brb", [O, TL], F32, "ExternalInput")
    Gr_d = ap("Gr", [O, 1], F32, "ExternalInput")
    Bbr_d = ap("Bbr", [O, 1], F32, "ExternalInput")
    epack_d = ap("epack", [1, NBM * NBM], F16, "ExternalInput")
    sel_d = ap("sel", [NBM, NBM * O], F16, "ExternalInput")
    out_d = ap("out", [O, B], F32, "ExternalOutput")

    with tile.TileContext(nc) as tc:
        import contextlib
        with contextlib.ExitStack() as ctx:
            pool = ctx.enter_context(tc.tile_pool(name="main", bufs=1))
            xpool = ctx.enter_context(tc.tile_pool(name="xs", bufs=6))
            stage = ctx.enter_context(tc.tile_pool(name="stage", bufs=2))
            ybufp = ctx.enter_context(tc.tile_pool(name="ybufp", bufs=4))
            yhatp = ctx.enter_context(tc.tile_pool(name="yhatp", bufs=4))
            ps1 = ctx.enter_context(tc.tile_pool(name="ps1", bufs=2, space="PSUM"))
            dram = ctx.enter_context(tc.tile_pool(name="dram", bufs=1, space="DRAM"))
            scr = ctx.enter_context(tc.tile_pool(name="scr", bufs=2))

            y_dram = [dram.tile([128, NCHUNK, NCOL], YDT[i], tag=f"ydram{i}", name=f"ydram{i}")
                      for i in range(2)]

            Srng = pool.tile([128, NCHUNK, SRING], F16, tag="Srng", name="Srng")
            y3 = pool.tile([O, NCOL], F32, tag="y3", name="y3")
            w1_sb = [[pool.tile([kk, 128], F16, tag=f"w1_{k}_{m}", name=f"w1_{k}_{m}")
                      for m in range(NCHUNK)] for k, (k0, kk) in enumerate(KJ)]
            for k, (k0, kk) in enumerate(KJ):
                for m in range(NCHUNK):
                    nc.sync.dma_start(w1_sb[k][m][:], w1t_d[k0:k0 + kk, m * 128:(m + 1) * 128])
            w2_sb = [[pool.tile([128, 128], F16, tag=f"w2_{k}_{m}", name=f"w2_{k}_{m}")
                      for m in range(NCHUNK)] for k in range(NCHUNK)]
            wr_sb = [pool.tile([128, O], F16, tag=f"wr_{k}", name=f"wr_{k}") for k in range(NCHUNK)]
            nbf = [pool.tile([128, NCHUNK, B], ST[i], tag=f"nbf{i}", name=f"nbf{i}") for i in range(2)]
            alpha_sb = pool.tile([128, 1], F32, tag="alpha", name="alpha")
            Gl = [pool.tile([128, NCHUNK], F32, tag=f"G_{i}", name=f"G_{i}") for i in range(2)]
            Bbl = [pool.tile([128, NCHUNK], F32, tag=f"Bb_{i}", name=f"Bb_{i}") for i in range(2)]
            brb_sb = pool.tile([O, TL], F32, tag="brb", name="brb")
            Gr_sb = pool.tile([O, 1], F32, tag="Gr", name="Gr")
            Bbr_sb = pool.tile([O, 1], F32, tag="Bbr", name="Bbr")

            def load_params():
                # ride the scalar-engine DMA queue so the x stream on the
                # sync queue is never head-blocked by this burst
                for k in range(NCHUNK):
                    for m in range(NCHUNK):
                        nc.scalar.dma_start(w2_sb[k][m][:], w2t_d[k * 128:(k + 1) * 128, m * 128:(m + 1) * 128])
                for k in range(NCHUNK):
                    nc.scalar.dma_start(wr_sb[k][:], wrt_d[k * 128:(k + 1) * 128, :])
                nc.scalar.dma_start(nbf[0][:].rearrange("p c b -> p (c b)"), nbf1_d)
                nc.scalar.dma_start(nbf[1][:].rearrange("p c b -> p (c b)"), nbf2_d)
                nc.scalar.dma_start(alpha_sb[:], alpha_d)
                nc.scalar.dma_start(Gl[0][:], G1_d); nc.scalar.dma_start(Bbl[0][:], Bb1_d)
                nc.scalar.dma_start(Gl[1][:], G2_d); nc.scalar.dma_start(Bbl[1][:], Bb2_d)
                nc.scalar.dma_start(brb_sb[:], brb_d)
                nc.scalar.dma_start(Gr_sb[:], Gr_d); nc.scalar.dma_start(Bbr_sb[:], Bbr_d)

            sums = [pool.tile([128, NCHUNK, NBM], F32, tag=f"sums{i}", name=f"sums{i}") for i in range(2)]
            sqs = [pool.tile([128, NCHUNK, NBM], F32, tag=f"sqs{i}", name=f"sqs{i}") for i in range(2)]
            sumr = pool.tile([O, NBM], F32, tag="sumr", name="sumr")
            sqr = pool.tile([O, NBM], F32, tag="sqr", name="sqr")

            Uq = [pool.tile([128, NCHUNK, NPARL[i], B], ST[i], tag=f"Uq{i}", name=f"Uq{i}")
                  for i in range(2)]
            Wst = [pool.tile([128, NCHUNK, B], ST[i], tag=f"Wst{i}", name=f"Wst{i}")
                   for i in range(2)]
            negone = pool.tile([128, 1], F32, tag="negone", name="negone")
            nc.gpsimd.memset(negone[:], -1.0)

            # ---------------- BN helpers ----------------
            def bn_coeffs(li, stats_sb, nch, parts, Gt, Bbt):
                m_ = pool.tile([parts, nch], F32, tag=f"mean{li}", name=f"mean{li}")
                v_ = pool.tile([parts, nch], F32, tag=f"var{li}", name=f"var{li}")
                t_ = pool.tile([parts, nch], F32, tag=f"tmp{li}", name=f"tmp{li}")
                c0 = pool.tile([parts, nch], F32, tag=f"c0_{li}", name=f"c0_{li}")
                c1 = pool.tile([parts, nch], F32, tag=f"c1_{li}", name=f"c1_{li}")
                inv_n = 1.0 / NTOT
                nc.vector.tensor_scalar(m_[:], stats_sb[:, 0:nch], inv_n, None, AOT.mult)
                nc.vector.tensor_scalar(v_[:], stats_sb[:, nch:2 * nch], inv_n, None, AOT.mult)
                nc.vector.tensor_tensor(t_[:], m_[:], m_[:], AOT.mult)
                nc.vector.tensor_tensor(v_[:], v_[:], t_[:], AOT.subtract)
                nc.vector.tensor_scalar(v_[:], v_[:], EPS, None, AOT.add)
                nc.scalar.sqrt(v_[:], v_[:])
                nc.vector.reciprocal(v_[:], v_[:])
                nc.vector.tensor_tensor(c0[:], v_[:], Gt[:], AOT.mult)
                nc.vector.tensor_tensor(t_[:], m_[:], c0[:], AOT.mult)
                nc.vector.tensor_tensor(c1[:], Bbt[:], t_[:], AOT.subtract)
                c0w = pool.tile([parts, nch], F32, tag=f"c0w_{li}", name=f"c0w_{li}")
                c1w = pool.tile([parts, nch], F32, tag=f"c1w_{li}", name=f"c1w_{li}")
                asl = alpha_sb[0:parts, :]
                nc.vector.tensor_scalar(c0w[:], c0[:], asl, None, AOT.mult)
                nc.vector.tensor_scalar(c1w[:], c1[:], asl, None, AOT.mult)
                return c0, c1, c0w, c1w

            def allreduce(sb_tile, parts, width, tag):
                din = dram.tile([parts, width], F32, tag=f"cin{tag}", name=f"cin{tag}")
                dout = dram.tile([parts, width], F32, tag=f"cout{tag}", name=f"cout{tag}")
                g = pool.tile([parts, width], F32, tag=f"gst{tag}", name=f"gst{tag}")
                nc.scalar.dma_start(din[:], sb_tile[:])
                nc.gpsimd.collective_compute(
                    "AllReduce", AOT.add,
                    replica_groups=[list(range(NC))],
                    ins=[din.opt()], outs=[dout.opt()],
                )
                return (g, dout)

            def stats_ar(li, sub, parts, nch, sums_t, sqs_t, lo, hi):
                st = pool.tile([parts, 2 * nch], F32, tag=f"st{sub}{li}", name=f"st{sub}{li}")
                nc.vector.tensor_reduce(st[:, 0:nch],
                                        sums_t[:, :, lo:hi] if nch > 1 else sums_t[:, lo:hi],
                                        mybir.AxisListType.X, AOT.add)
                nc.vector.tensor_reduce(st[:, nch:],
                                        sqs_t[:, :, lo:hi] if nch > 1 else sqs_t[:, lo:hi],
                                        mybir.AxisListType.X, AOT.add)
                return allreduce(st, parts, 2 * nch, f"{sub}{li}")

            def finish_stats(li, parts, nch, ga, Gt, Bbt):
                g, d_ = ga
                nc.scalar.dma_start(g[:], d_[:])
                return bn_coeffs(li, g, nch, parts, Gt, Bbt)

            # ---------------- y prefetch + ACT fold ----------------
            _yb = {}   # s -> raw f32 ybuf tile
            _yh = {}   # s -> folded scan-input tile

            def prefetch_y(li, s):
                t_ = ybufp.tile([128, NCHUNK, 512], F32, tag="ybuf", name="ybuf")
                nc.sync.dma_start(t_[:], y_dram[li][:, :, s * 512:(s + 1) * 512])
                _yb[s] = t_

            def fold_block(li, s, c0, c1, c0w, c1w):
                a0, a1 = (c0w, c1w) if s < NBW else (c0, c1)
                yb = _yb.pop(s)
                if ST[li] == F32:
                    yh = yb  # in-place fold
                else:
                    yh = yhatp.tile([128, NCHUNK, 512], ST[li], tag="yhat", name="yhat")
                for m in range(NCHUNK):
                    nc.scalar.activation(yh[:, m, :], yb[:, m, :], AFT.Identity,
                                         bias=a1[:, m:m + 1], scale=a0[:, m:m + 1])
                _yh[s] = yh

            # ================ LAYER 1 matmul ================
            ar1 = [None]

            def l1_block(s):
                cols = slice(s * 512, (s + 1) * 512)
                psl = [ps1.tile([128, 512], F32, tag=f"ps_m{m}", name=f"ps_m{m}")
                       for m in range(NCHUNK)]
                rhs = []
                for k, (k0, kk) in enumerate(KJ):
                    xt = xpool.tile([128, 512], F16, tag="xstream", name="xs")
                    nc.sync.dma_start(xt[:kk, :], x_d[k0:k0 + kk, cols])
                    rhs.append(xt[:kk, :])
                for m in range(NCHUNK):
                    for k in range(len(KJ)):
                        nc.tensor.matmul(psl[m][:], w1_sb[k][m][:], rhs[k],
                                         start=(k == 0), stop=(k == len(KJ) - 1))
                st_t = stage.tile([128, NCHUNK, 512], F32, tag="ystage", name="ystage")
                main = s >= NBW
                for m in range(NCHUNK):
                    if main:
                        nc.scalar.activation(st_t[:, m, :], psl[m][:], AFT.Copy,
                                             accum_out=sums[0][:, m, s - NBW:s - NBW + 1])
                        sc = scr.tile([128, 512], BF16, tag="sq_scratch", name="sq")
                        nc.scalar.activation(sc[:], psl[m][:], AFT.Square,
                                             accum_out=sqs[0][:, m, s - NBW:s - NBW + 1])
                    else:
                        nc.scalar.activation(st_t[:, m, :], psl[m][:], AFT.Copy)
                nc.sync.dma_start(y_dram[0][:, :, cols], st_t[:])

            for i, s in enumerate(list(range(NBW, NBLK)) + list(range(NBW))):
                l1_block(s)
                if i == 0:
                    load_params()
                if i == NBM - 1:
                    ar1[0] = stats_ar(0, "a", 128, NCHUNK, sums[0], sqs[0], 0, NBM)
            c0_1, c1_1, c0w_1, c1w_1 = finish_stats(0, 128, NCHUNK, ar1[0], Gl[0], Bbl[0])

            # ---------------- LIF scan emitter ----------------
            def lif_scan(li, c0, c1, c0w, c1w, mm_batch_cb, drain_cb, ndrain):
                npar = NPARL[li]
                sw = SWL[li]
                nbv = nbf[li]
                U = Uq[li]
                W = Wst[li]
                for s in range(4):
                    prefetch_y(li, s)
                fold_block(li, 0, c0, c1, c0w, c1w)
                fold_block(li, 1, c0, c1, c0w, c1w)
                fold_block(li, 2, c0, c1, c0w, c1w)
                pend = []
                for s in range(NBLK):
                    if s + 4 < NBLK:
                        prefetch_y(li, s + 4)
                    if s + 3 < NBLK:
                        fold_block(li, s + 3, c0, c1, c0w, c1w)
                    yh = _yh.pop(s)
                    for tt in range(TBLK):
                        t = s * TBLK + tt
                        p = t % npar
                        bs = slice(tt * B, (tt + 1) * B)
                        Ua = U[:, 0:2, p, :]
                        Ub = U[:, 2:4, p, :]
                        if t == 0:
                            # state is implicitly zero: U = yhat
                            nc.vector.tensor_scalar(Ua, yh[:, 0:2, bs], 1.0, None, AOT.mult)
                            nc.vector.tensor_scalar(Ub, yh[:, 2:4, bs], 1.0, None, AOT.mult)
                        else:
                            nc.vector.tensor_tensor(Ua, W[:, 0:2, :], nbv[:, 0:2, :], AOT.mult)
                            nc.vector.tensor_tensor(Ub, W[:, 2:4, :], nbv[:, 2:4, :], AOT.mult)
                            nc.vector.tensor_tensor(Ua, Ua, yh[:, 0:2, bs], AOT.add)
                            nc.vector.tensor_tensor(Ub, Ub, yh[:, 2:4, bs], AOT.add)
                        nc.vector.scalar_tensor_tensor(W[:, 0:2, :], Ua, 1.0, Ua,
                                                       AOT.is_gt, AOT.subtract)
                        nc.vector.scalar_tensor_tensor(W[:, 2:4, :], Ub, 1.0, Ub,
                                                       AOT.is_gt, AOT.subtract)
                        if t % sw == sw - 1:
                            q0 = sw * ((t // sw) % 2)
                            usrc = U[:, :, q0:q0 + sw, :].rearrange("p c q b -> p c (q b)")
                            r0 = ((t - sw + 1) * B) % SRING
                            sdst = Srng[:, :, r0:r0 + sw * B]
                            nc.scalar.activation(sdst, usrc, AFT.Sign, bias=negone[:])
                        for _ in range(ndrain):
                            if pend:
                                pend.pop(0)()
                    if s % 2 == 1:
                        mm_batch_cb(s - 1, s)
                        pend.extend(drain_cb(s - 1))
                        pend.extend(drain_cb(s))
                while pend:
                    pend.pop(0)()

            # ---- layer-2 matmul + drains
            _ps = {}
            ar2 = [None, None]

            def l2_mm_batch(sa, sb):
                for s in (sa, sb):
                    r0 = (s * 512) % SRING
                    psl = [ps1.tile([128, 512], F32, tag=f"ps_m{m}", name=f"ps_m{m}")
                           for m in range(NCHUNK)]
                    for m in range(NCHUNK):
                        for k in range(NCHUNK):
                            nc.tensor.matmul(psl[m][:], w2_sb[k][m][:],
                                             Srng[:, k, r0:r0 + 512],
                                             start=(k == 0), stop=(k == NCHUNK - 1))
                    _ps[s] = psl

            _st = {}

            def l2_drain(s):
                ops = []
                main = s >= NBW

                def mk(m, sq):
                    def f():
                        if s not in _st:
                            _st[s] = stage.tile([128, NCHUNK, 512], F32, tag="ystage", name="ystage")
                        st_t = _st[s]
                        if not sq:
                            if main:
                                nc.scalar.activation(st_t[:, m, :], _ps[s][m][:], AFT.Copy,
                                                     accum_out=sums[1][:, m, s - NBW:s - NBW + 1])
                            else:
                                nc.scalar.activation(st_t[:, m, :], _ps[s][m][:], AFT.Copy)
                        else:
                            sc = scr.tile([128, 512], BF16, tag="sq_scratch", name="sq")
                            nc.scalar.activation(sc[:], _ps[s][m][:], AFT.Square,
                                                 accum_out=sqs[1][:, m, s - NBW:s - NBW + 1])
                    return f

                for m in range(NCHUNK):
                    ops.append(mk(m, False))
                    if main:
                        ops.append(mk(m, True))

                def fin():
                    st_t = _st.pop(s)
                    _ps.pop(s)
                    nc.sync.dma_start(y_dram[1][:, :, s * 512:(s + 1) * 512], st_t[:])
                ops.append(fin)
                return ops

            lif_scan(0, c0_1, c1_1, c0w_1, c1w_1, l2_mm_batch, l2_drain, ndrain=6)
            ar2[0] = stats_ar(1, "a", 128, NCHUNK, sums[1], sqs[1], 0, NBM)
            c0_2, c1_2, c0w_2, c1w_2 = finish_stats(1, 128, NCHUNK, ar2[0], Gl[1], Bbl[1])

            # ---- layer-3 (readout) matmul + drains
            ar3 = [None, None]

            def l3_mm_batch(sa, sb):
                for s in (sa, sb):
                    r0 = (s * 512) % SRING
                    ps = ps1.tile([O, 512], F32, tag="ps_m0", name="ps_r")
                    for k in range(NCHUNK):
                        nc.tensor.matmul(ps[:], wr_sb[k][:], Srng[:, k, r0:r0 + 512],
                                         start=(k == 0), stop=(k == NCHUNK - 1))
                    _ps[s] = ps

            def l3_drain(s):
                cols = slice(s * 512, (s + 1) * 512)
                main = s >= NBW
                ops = []

                def cp():
                    if main:
                        nc.scalar.activation(y3[:, cols], _ps[s][:], AFT.Copy,
                                             accum_out=sumr[:, s - NBW:s - NBW + 1])
                    else:
                        nc.scalar.activation(y3[:, cols], _ps[s][:], AFT.Copy)
                ops.append(cp)

                def sq():
                    if main:
                        sc = scr.tile([O, 512], BF16, tag="sq3_scratch", name="sq3")
                        nc.scalar.activation(sc[:], _ps[s][:], AFT.Square,
                                             accum_out=sqr[:, s - NBW:s - NBW + 1])
                    _ps.pop(s)
                ops.append(sq)
                return ops

            lif_scan(1, c0_2, c1_2, c0w_2, c1w_2, l3_mm_batch, l3_drain, ndrain=3)
            ar3[0] = stats_ar(2, "a", O, 1, sumr, sqr, 0, NBM)
            c0_r, c1_r, c0w_r, c1w_r = finish_stats(2, O, 1, ar3[0], Gr_sb, Bbr_sb)

            # ================ READOUT ================
            nc.vector.tensor_scalar(y3[:, 0:NBW * 512], y3[:, 0:NBW * 512],
                                    c0w_r[:, 0:1], c1w_r[:, 0:1], AOT.mult, AOT.add)
            for q in range(2):
                sl = slice(NBW * 512 + q * 4096, NBW * 512 + (q + 1) * 4096)
                nc.vector.tensor_scalar(y3[:, sl], y3[:, sl],
                                        c0_r[:, 0:1], c1_r[:, 0:1], AOT.mult, AOT.add)
            y3v = y3[:].rearrange("p (t b) -> p t b", b=B)
            for b in range(B):
                sl = y3v[:, :, b]
                nc.vector.tensor_tensor_scan(sl, brb_sb[:], sl, 0.0, AOT.mult, AOT.add)

            ones_k20 = pool.tile([O, 1], F16, tag="ones_k20", name="ones_k20")
            nc.gpsimd.memset(ones_k20[:], 1.0)
            Epack = pool.tile([1, NBM * NBM], F16, tag="Epack", name="Epack")
            nc.sync.dma_start(Epack[:], epack_d)
            Sel = pool.tile([NBM, NBM * O], F16, tag="Sel", name="Sel")
            nc.sync.dma_start(Sel[:], sel_d)
            zall_ps = ps1.tile([NBM, 512], F32, tag="ps_m1", name="ps_zall")
            M0 = NBW * 512
            for n in range(NBM):
                cols = slice(M0 + n * 512, M0 + (n + 1) * 512)
                En = scr.tile([O, 512], F16, tag="En", name="En")
                nc.scalar.activation(En[:], y3[:, cols], AFT.Exp)
                psz = ps1.tile([1, 512], F32, tag="ps_m2", name="ps_z")
                nc.tensor.matmul(psz[:], ones_k20[:], En[:], start=True, stop=True)
                zsb = scr.tile([1, 512], F16, tag="zsb", name="zsb")
                nc.scalar.copy(zsb[:], psz[:])
                nc.tensor.matmul(zall_ps[:], Epack[0:1, n * NBM:(n + 1) * NBM],
                                 zsb[:], start=(n == 0), stop=(n == NBM - 1))
            Rall = pool.tile([NBM, 512], F16, tag="Rall", name="Rall")
            with nc.allow_low_precision(reason="softmax denominator, fp16 ok"):
                nc.vector.reciprocal(Rall[:], zall_ps[:])
            for n in range(NBM):
                cols = slice(M0 + n * 512, M0 + (n + 1) * 512)
                En = scr.tile([O, 512], F16, tag="En", name="En")
                nc.scalar.activation(En[:], y3[:, cols], AFT.Exp)
                psb = ps1.tile([O, 512], F32, tag="ps_m3", name="ps_b")
                nc.tensor.matmul(psb[:], Sel[:, n * O:(n + 1) * O], Rall[:],
                                 start=True, stop=True)
                nc.vector.tensor_tensor(y3[:, cols], En[:], psb[:], AOT.mult)
            res = pool.tile([O, B], F32, tag="res", name="res")
            accv = y3[:, M0:].rearrange("p (t b) -> p b t", b=B)
            nc.vector.tensor_reduce(res[:, 0:B // 2], accv[:, 0:B // 2, :],
                                    mybir.AxisListType.X, AOT.add)
            nc.vector.tensor_reduce(res[:, B // 2:], accv[:, B // 2:, :],
                                    mybir.AxisListType.X, AOT.add)
            nc.sync.dma_start(out_d, res[:])

    nc.compile()
    return nc


def _host_prep(inputs):
    f32 = np.float32
    x = np.asarray(inputs["x"], f32)
    sig = lambda v: (1.0 / (1.0 + np.exp(-np.asarray(v, np.float64)))).astype(f32)

    def packed(vec):
        return np.ascontiguousarray(np.asarray(vec, f32).reshape(NCHUNK, 128).T)

    beta1, beta2, betar = sig(inputs["beta1"]), sig(inputs["beta2"]), sig(inputs["betar"])

    def nbfull(beta, st):
        p = packed(-beta)
        dt = np.float16 if st == F16 else f32
        return np.ascontiguousarray(
            np.repeat(p[:, :, None], B, axis=2).reshape(128, NCHUNK * B)).astype(dt)

    com = {
        "w1t": np.ascontiguousarray(np.asarray(inputs["W1"], f32).T).astype(np.float16),
        "w2t": np.ascontiguousarray(np.asarray(inputs["W2"], f32).T * 0.5).astype(np.float16),
        "wrt": np.ascontiguousarray(np.asarray(inputs["Wr"], f32).T * 0.5).astype(np.float16),
        "nbf1": nbfull(beta1, ST[0]),
        "nbf2": nbfull(beta2, ST[1]),
        "G1": packed(np.asarray(inputs["g1"], f32) * (1 - beta1)),
        "Bb1": packed(np.asarray(inputs["b1"], f32) * (1 - beta1)),
        "G2": packed(np.asarray(inputs["g2"], f32) * (1 - beta2)),
        "Bb2": packed(np.asarray(inputs["b2"], f32) * (1 - beta2)),
        "brb": np.ascontiguousarray(np.repeat(betar[:, None], TL, axis=1)),
        "Gr": np.ascontiguousarray((np.asarray(inputs["gr"], f32) * (1 - betar))[:, None]),
        "Bbr": np.ascontiguousarray((np.asarray(inputs["br"], f32) * (1 - betar))[:, None]),
        "epack": np.eye(NBM, dtype=np.float16).reshape(1, NBM * NBM),
        "sel": np.ascontiguousarray(
            np.repeat(np.eye(NBM, dtype=np.float16)[:, :, None], O, axis=2).reshape(NBM, NBM * O)),
    }
    in_maps = []
    for c in range(NC):
        t0 = c * SH
        lo = t0 - WU
        src0 = max(0, lo)
        xc = np.zeros((J, TL, B), np.float16)
        xc[:, (src0 - lo):, :] = x[:, src0:t0 + SH, :].transpose(2, 1, 0).astype(np.float16)
        m = dict(com)
        m["x"] = np.ascontiguousarray(xc.reshape(J, NCOL))
        m["alpha"] = np.full((128, 1), 0.0 if c == 0 else 1.0, np.float32)
        in_maps.append(m)
    return in_maps


def kernel(**inputs):
    if "nc" not in _CACHE:
        _CACHE["nc"] = _build_program()
    nc = _CACHE["nc"]
    in_maps = _host_prep(inputs)
    res = run_bass_kernel_spmd(nc, in_maps, core_ids=list(range(NC)),
                               trace=bool(os.environ.get("BASS_TRACE_KERNEL")))
    _CACHE["last_result"] = res
    out = np.zeros((B, O), np.float32)
    for c in range(NC):
        out += res.results[c]["out"].T
    return out


if __name__ == "__main__":
    t0 = time.time()
    nc = _build_program()
    print(f"build+compile ok in {time.time()-t0:.1f}s")
